# revision 28
# baseline (speedup 1.0000x reference)
"""MiniBatchDiscrimination kernel, v5.1: fp8 DoubleRow everywhere it pays
+ triangle-trimmed diagonal block + 3-engine elementwise split.

Per core (row block c): columns = 4 partner blocks (c+1..c+4 mod 8) at
positions 0-3 and the OWN (diagonal) block last, cols 256:320.  Per row i
only cols [0, 257+i) are computed (j <= i within the diagonal block); the
j > i half comes from the column sums by symmetry (4 block pairs are
computed twice, once per orientation, so every core carries 5 real
blocks and no poison).

  M = x @ T on PE as fp8 DoubleRow matmuls: inputs are host-quantized
  x*ax, T*aT with ax, aT POWERS OF TWO, so the compensation beta =
  2^12/(ax aT) is exact in bf16 and fp8 and is folded into the selector
  weights at zero cost.  Mt'' tiles (bf16) = psum * 2^-12 (fixed scale).
  Per i:  d_t = Mt_t - Mt_t[:, 256+i] per tile:
    tiles 0-9   DVE  relu(d) -> bf16     (weight 2 beta, bf16 matmuls)
    tiles 10,11 ACT  |d| via Abs -> fp8  (weight beta, DR pair 0)
    tiles 12-14 Pool relu(d) -> fp8     (weight 2 beta, DR pairs 1-2)
    tile  15    DVE  relu(d) -> fp8     (weight 2 beta, DR pair 2)
  (the DVE/Pool TensorScalar ISA has no (subtract, abs_max), hence relu
  with |d| = 2 relu(d) - d and the S correction below; ACT's Abs is a
  real activation function.)
  Reduction on PE into one PSUM bank [64, W]: 10 bf16 matmuls + 3 fp8
  DoubleRow pair matmuls + 1 DR pair (-beta * S8, 0-weight dummy) that
  applies the -S_j correction.  S8 = fp8(sum_k Mt'' over relu tiles).
  The Exp bias +S_i is -nbeta*S8[:, 256+i] computed from the SAME fp8
  values, so D_ii == 0 exactly and exp(-0) = 1 lands in rowS/accS.
  Exp on ACT packs e tiles (2 rows each) with accum_out -> rowS.
  Column sums per 16-row chunk on PE: a zero-weight full-width matmul
  opens the PSUM accumulation group, then one pair matmul right after
  each odd row's Exp (ascending widths), accumulated into accS on DVE.

Host: mbd rows c = rowS_c + accS transposes from cores c-1..c-3 +
own-diagonal accS tail (+1 for odd rows' self term missing from the
pair-packed column sums, -1 for the double-counted E_ii, -1 for the
reference's self-similarity subtraction).
"""

import math

import numpy as np
import ml_dtypes
from contextlib import ExitStack

BATCH, IN_FEAT, OUT_FEAT, KERNEL_DIM = 512, 512, 64, 32
N_CORES = 8
ROWB = BATCH // N_CORES          # 64 rows of i per core
OK = OUT_FEAT * KERNEL_DIM       # 2048 flattened (o,k)
NT = OK // 128                   # 16 partition-tiles of (o,k)
NBLK = 5                         # column blocks per core
FD = NBLK * 64                   # 320
DIAG0 = (NBLK - 1) * 64          # 256: diagonal block start column

DVE_BF = (0, 1, 2, 3, 4, 5, 6, 7, 8, 9)
PAIRS = ((10, 11), (12, 13), (14, 15))
PROD = {10: "act", 11: "act", 12: "pool", 13: "pool", 14: "pool",
        15: "dve8"}
RELU_T = DVE_BF + tuple(t for t, e in PROD.items() if e != "act")
CHUNK = 16                       # i's per colsum chunk
SELW = OUT_FEAT
# iterations where tile 15 is produced on Pool instead of DVE (fractional
# load balance: DVE and Pool rates differ ~3.7x)
T15_POOL = frozenset(i for i in range(ROWB) if i % 6 == 3)
OUT_NAMES = ("rowsA", "rowsB", "acc0", "acc1", "acc2", "acc3")

_cache = {}


def _build_nc(split_waits=True):
    import concourse.bass as bass
    import concourse.mybir as mybir
    import concourse.tile as tile

    dt = mybir.dt
    AF = mybir.ActivationFunctionType
    OP = mybir.AluOpType
    DR = mybir.MatmulPerfMode.DoubleRow

    nc = bass.Bass("TRN2", target_bir_lowering=False, debug=False,
                   num_devices=N_CORES)

    # fp8 inputs for the DoubleRow M matmuls: pack q holds infeat chunks
    # (2q, 2q+1) interleaved in the free dim (the DR pair dimension)
    T_d = nc.dram_tensor("Tm", [2 * 128, 2 * OK], dt.float8e4,
                         kind="ExternalInput")
    xT_d = nc.dram_tensor("xT", [2 * 128, 2 * FD], dt.float8e4,
                          kind="ExternalInput")
    selb_d = nc.dram_tensor("selb", [128, len(DVE_BF) * SELW], dt.bfloat16,
                            kind="ExternalInput")
    seldr_d = nc.dram_tensor("seldr", [128, (len(PAIRS) + 1) * 128],
                             dt.float8e4, kind="ExternalInput")
    sel2_d = nc.dram_tensor("sel2", [128, OUT_FEAT], dt.bfloat16,
                            kind="ExternalInput")
    Ts_d = nc.dram_tensor("Ts", [2 * 128, 2 * SELW], dt.float8e4,
                          kind="ExternalInput")
    nbeta_d = nc.dram_tensor("nbeta", [OUT_FEAT, 1], dt.float32,
                             kind="ExternalInput")
    out_d = {}
    for nm in ("rowsA", "rowsB"):
        out_d[nm] = nc.dram_tensor(nm, [OUT_FEAT, ROWB // 2], dt.float32,
                                   kind="ExternalOutput")
    for ch in range(ROWB // CHUNK):
        out_d[f"acc{ch}"] = nc.dram_tensor(f"acc{ch}", [OUT_FEAT, FD],
                                           dt.float32,
                                           kind="ExternalOutput")

    with tile.TileContext(nc) as tc, ExitStack() as ctx:
        const = ctx.enter_context(tc.tile_pool(name="const", bufs=1))
        mtp = ctx.enter_context(tc.tile_pool(name="mt", bufs=NT))
        advp = ctx.enter_context(tc.tile_pool(name="adv", bufs=30))
        pairp = ctx.enter_context(tc.tile_pool(name="pair", bufs=9))
        ep = ctx.enter_context(tc.tile_pool(name="e", bufs=20))
        psda = ctx.enter_context(
            tc.tile_pool(name="psda", bufs=3, space=bass.MemorySpace.PSUM))
        psc = ctx.enter_context(
            tc.tile_pool(name="psc", bufs=2, space=bass.MemorySpace.PSUM))

        # ---- input DMAs: first-needed first, spread over the SP/ACT/DVE
        # HWDGE queues so the first M matmul can start after ~1.5us ----
        Tsb, xsb = [], []
        for q in range(2):
            t_ = const.tile([128, 2 * OK], dt.float8e4, tag=f"T{q}")
            Tsb.append(t_)
            x_ = const.tile([128, 2 * FD], dt.float8e4, tag=f"x{q}")
            xsb.append(x_)
        nc.sync.dma_start(Tsb[0][:], T_d[0:128, :])
        nc.sync.dma_start(xsb[0][:], xT_d[0:128, :])
        nc.scalar.dma_start(Tsb[1][:], T_d[128:256, :])
        nc.scalar.dma_start(xsb[1][:], xT_d[128:256, :])
        selb = const.tile([128, len(DVE_BF) * SELW], dt.bfloat16, tag="selb")
        nc.sync.dma_start(selb[:], selb_d[:])
        seldr = const.tile([128, (len(PAIRS) + 1) * 128], dt.float8e4,
                           tag="seldr")
        nc.sync.dma_start(seldr[:], seldr_d[:])
        sel2 = const.tile([128, OUT_FEAT], dt.bfloat16, tag="sel2")
        nc.sync.dma_start(sel2[:], sel2_d[:])
        Ts = []
        for q in range(2):
            ts_ = const.tile([128, 2 * SELW], dt.float8e4, tag=f"Ts{q}")
            nc.sync.dma_start(ts_[:], Ts_d[q * 128:(q + 1) * 128, :])
            Ts.append(ts_)
        nbeta = const.tile([OUT_FEAT, 1], dt.float32, tag="nbeta")
        nc.sync.dma_start(nbeta[:], nbeta_d[:])

        rowS = const.tile([OUT_FEAT, ROWB], dt.float32, tag="rowS")
        accsb = []
        for ch in range(ROWB // CHUNK):
            a_ = const.tile([OUT_FEAT, FD], dt.float32, tag=f"accsb{ch}")
            accsb.append(a_)
        # fp32 image of the (rounded) bf16 diagonal columns: the subtracted
        # scalar exactly equals the tile value, so d_ii == 0 exactly
        mcol = const.tile([128, NT * ROWB], dt.float32, tag="mcol")
        # zero weights: opens each colsum PSUM group at full width
        zw = const.tile([128, SELW], dt.bfloat16, tag="zw")
        nc.vector.memset(zw[:], 0.0)

        # ---- M preamble: Mt''[(o,k), j] bf16 tiles via DR matmuls ----
        mts = []
        with tc.tile_pool(name="psm", bufs=2,
                          space=bass.MemorySpace.PSUM) as psm:
            for t in range(NT):
                ps = psm.tile([128, FD], dt.float32, tag="psm",
                              name=f"psm_{t}")
                for q in range(2):
                    nc.tensor.matmul(
                        ps[:],
                        Tsb[q][:].rearrange("p (two m) -> p two m",
                                            two=2)[:, :,
                                                   t * 128:(t + 1) * 128],
                        xsb[q][:].rearrange("p (two n) -> p two n", two=2),
                        start=(q == 0), stop=(q == 1), perf_mode=DR)
                mt_t = mtp.tile([128, FD], dt.bfloat16, tag="mt",
                                name=f"mt_{t}")
                # descale by the FIXED 2^-12 (the data-dependent remainder
                # beta lives in the selector weights)
                if t < 8:
                    nc.vector.tensor_scalar(mt_t[:], ps[:], 2.0 ** -12,
                                            None, op0=OP.mult)
                else:
                    nc.scalar.activation(mt_t[:], ps[:], AF.Copy,
                                         scale=2.0 ** -12)
                nc.gpsimd.tensor_copy(mcol[:, t * ROWB:(t + 1) * ROWB],
                                      mt_t[:, DIAG0:FD])
                mts.append(mt_t)
            # S via the precontracted Ts = sum_k T (relu tiles' o only):
            # S8 = fp8(psS * 2^-12); the -betaS*S8 matmul and the Exp bias
            # +betaS*S8_i read the SAME fp8 values, so D_ii == 0 exactly
            psS = psc.tile([OUT_FEAT, FD], dt.float32, tag="psc",
                           name="psS")
            for q in range(2):
                nc.tensor.matmul(
                    psS[:],
                    Ts[q][:].rearrange("p (two m) -> p two m", two=2),
                    xsb[q][:].rearrange("p (two n) -> p two n", two=2),
                    start=(q == 0), stop=(q == 1), perf_mode=DR)
            S8 = const.tile([OUT_FEAT, 2 * FD], dt.float8e4, tag="S8")
            nc.scalar.activation(S8[:, 0:FD], psS[:], AF.Copy,
                                 scale=2.0 ** -12)
            nc.scalar.activation(S8[:, FD:2 * FD], psS[:], AF.Copy,
                                 scale=2.0 ** -12)
            Sneg = const.tile([OUT_FEAT, ROWB], dt.float32, tag="Sneg")
            nc.vector.tensor_scalar(Sneg[:], S8[:, DIAG0:FD], nbeta[:],
                                    None, op0=OP.mult)

        e_lists = [[] for _ in range(ROWB // CHUNK)]
        pend_exp = None
        pc_cur = [None]

        def emit_exp(i, ps_i):
            W = DIAG0 + i + 1
            ch = i // CHUNK
            if i % 2 == 0:
                e_t = ep.tile([128, FD], dt.bfloat16, tag="e",
                              name=f"e_{i}")
                e_lists[ch].append(e_t)
            half = e_lists[ch][-1][(i % 2) * OUT_FEAT:
                                   (i % 2 + 1) * OUT_FEAT, 0:W]
            nc.scalar.activation(half, ps_i[:, 0:W], AF.Exp,
                                 scale=-1.0, bias=Sneg[:, i:i + 1],
                                 accum_out=rowS[:, i:i + 1])
            if i == ROWB // 2 - 1:
                nc.sync.dma_start(out_d["rowsA"][:],
                                  rowS[:, 0:ROWB // 2])
            elif i == ROWB - 1:
                nc.sync.dma_start(out_d["rowsB"][:],
                                  rowS[:, ROWB // 2:ROWB])
            if i % 2 == 1:
                # colsum pair matmul for rows (i-1, i), ascending widths;
                # a zero-weight full-width matmul opened the group so the
                # PSUM zero region is already cleared
                m = (i % CHUNK) // 2
                if m == 0:
                    pc_cur[0] = psc.tile([OUT_FEAT, FD], dt.float32,
                                         tag="psc", name=f"psc_{ch}")
                    nc.tensor.matmul(pc_cur[0][:], zw[:, 0:SELW],
                                     mts[0][:], start=True, stop=False)
                Wp = DIAG0 + CHUNK * ch + 2 * m + 1
                nc.tensor.matmul(pc_cur[0][:, 0:Wp], sel2[:],
                                 e_lists[ch][m][:, 0:Wp],
                                 start=False, stop=(m == CHUNK // 2 - 1))
                if m == CHUNK // 2 - 1:
                    # stage the chunk's column sums to SBUF (ACT) and DMA
                    # out on the idle SP queue; the zero-weight opener
                    # zeroed the tail columns
                    nc.scalar.activation(accsb[ch][:], pc_cur[0][:],
                                         AF.Copy)
                    nc.sync.dma_start(out_d[f"acc{ch}"][:], accsb[ch][:])

        for i in range(ROWB):
            W = DIAG0 + i + 1
            ps_i = psda.tile([OUT_FEAT, FD], dt.float32, tag="psda",
                             name=f"psda_{i}")
            # ---- elementwise tiles ----
            ads = {}
            pbs = []
            for pi, (ta, tb) in enumerate(PAIRS):
                pb = pairp.tile([128, 2 * FD], dt.float8e4, tag="pb",
                                name=f"pb_{i}_{pi}")
                pbs.append(pb)
                ads[ta] = pb[:, 0:W]
                ads[tb] = pb[:, FD:FD + W]
            # DVE fp8 half first so its consumer (last DR matmul) is ready
            for t, eng in PROD.items():
                if eng != "dve8":
                    continue
                sc = mcol[:, t * ROWB + i:t * ROWB + i + 1]
                e_ = nc.gpsimd if i in T15_POOL else nc.vector
                e_.tensor_scalar(ads[t], mts[t][:, 0:W], sc, 0.0,
                                 op0=OP.subtract, op1=OP.max)
            for t in DVE_BF:
                sc = mcol[:, t * ROWB + i:t * ROWB + i + 1]
                ad_t = advp.tile([128, FD], dt.bfloat16, tag="adv",
                                 name=f"ad_{i}_{t}")
                nc.vector.tensor_scalar(ad_t[:, 0:W], mts[t][:, 0:W], sc,
                                        0.0, op0=OP.subtract, op1=OP.max)
                ads[t] = ad_t[:, 0:W]
            for t, eng in PROD.items():
                sc = mcol[:, t * ROWB + i:t * ROWB + i + 1]
                if eng == "act":
                    nc.scalar.activation(ads[t], mts[t][:, 0:W], AF.Abs,
                                         bias=sc, scale=-1.0)
                elif eng == "pool":
                    nc.gpsimd.tensor_scalar(ads[t], mts[t][:, 0:W], sc, 0.0,
                                            op0=OP.subtract, op1=OP.max)
            # ---- reduction matmuls into one PSUM bank ----
            for m, t in enumerate(DVE_BF):
                nc.tensor.matmul(ps_i[:, 0:W],
                                 selb[:, m * SELW:(m + 1) * SELW],
                                 ads[t], start=(m == 0), stop=False)
            for pi in range(len(PAIRS)):
                nc.tensor.matmul(
                    ps_i[:, 0:W],
                    seldr[:, pi * 128:(pi + 1) * 128].rearrange(
                        "p (two m) -> p two m", two=2),
                    pbs[pi][:, 0:2 * FD].rearrange(
                        "p (two n) -> p two n", two=2)[:, :, 0:W],
                    start=False, stop=False,
                    perf_mode=DR)
            # -S_j correction: DR pair (-beta * S8, zero-weight dummy)
            npi = len(PAIRS)
            nc.tensor.matmul(
                ps_i[:, 0:W],
                seldr[0:OUT_FEAT,
                      npi * 128:(npi + 1) * 128].rearrange(
                    "p (two m) -> p two m", two=2),
                S8[:, 0:2 * FD].rearrange(
                    "p (two n) -> p two n", two=2)[:, :, 0:W],
                start=False, stop=True, perf_mode=DR)
            # ---- software-pipelined Exp (+ inline colsum pairs) ----
            if pend_exp is not None:
                emit_exp(*pend_exp)
            pend_exp = (i, ps_i)
        emit_exp(*pend_exp)

    if split_waits:
        _split_multiwaits(nc, mybir)
    return nc


def _split_multiwaits(nc, mybir):
    """Walrus encodes at most ONE sync-wait command per instruction. Split
    any instruction with more into a chain of single-wait Drain carriers on
    the same engine, inserted immediately before it."""
    n = 0
    for fn in nc.m.functions:
        for bb in fn.blocks:
            new_insts = []
            for inst in bb.instructions:
                si = getattr(inst, "sync_info", None)
                if si is not None and si.on_wait and len(si.on_wait) > 1:
                    waits = list(si.on_wait)
                    for w in waits[:-1]:
                        carrier = mybir.InstDrain(
                            name=f"splitw_{n}", engine=inst.engine,
                            ins=[], outs=[],
                            sync_info=mybir.SyncInfo(on_wait=[w],
                                                     on_update=[]))
                        new_insts.append(carrier)
                        n += 1
                    inst.sync_info = mybir.SyncInfo(
                        on_wait=[waits[-1]], on_update=list(si.on_update))
                new_insts.append(inst)
            if n:
                bb.instructions = new_insts


def _pow2_scale(m, target=200.0):
    """Largest power of two a with m * a <= target."""
    if not np.isfinite(m) or m <= 0:
        return 1.0
    return 2.0 ** math.floor(math.log2(target / m))


def _selb_host(beta):
    sel = np.zeros((128, len(DVE_BF) * SELW), dtype=np.float32)
    for m, t in enumerate(DVE_BF):
        for g in range(4):
            sel[32 * g:32 * (g + 1), m * SELW + 4 * t + g] = 2.0 * beta
    return sel.astype(ml_dtypes.bfloat16)


def _seldr_host(beta, betaS):
    sel = np.zeros((128, (len(PAIRS) + 1) * 128), dtype=np.float32)
    for pi, pair in enumerate(PAIRS):
        for h, t in enumerate(pair):
            v = beta if PROD[t] == "act" else 2.0 * beta
            for g in range(4):
                sel[32 * g:32 * (g + 1),
                    pi * 128 + h * SELW + 4 * t + g] = v
    # pair 3: half0 = -betaS * I64 (the S correction), half1 = 0
    npi = len(PAIRS)
    sel[0:OUT_FEAT, npi * 128:npi * 128 + OUT_FEAT] = \
        -betaS * np.eye(OUT_FEAT, dtype=np.float32)
    return sel.astype(ml_dtypes.float8_e4m3)


def _Ts_host(T, aTs):
    """Precontracted T for the S matmuls: Ts[i, o] = sum_k T[i, o, k] for
    o's whose tile holds relu values, else 0; packed for DoubleRow."""
    Ts = T.sum(axis=2) * aTs                      # [IN_FEAT, OUT_FEAT]
    mask = np.zeros(OUT_FEAT, dtype=np.float32)
    for t in RELU_T:
        mask[4 * t:4 * t + 4] = 1.0
    return _pack_pairs((Ts * mask).astype(ml_dtypes.float8_e4m3))


def _sel2_host():
    s = np.zeros((128, OUT_FEAT), dtype=np.float32)
    s[:OUT_FEAT, :] = np.eye(OUT_FEAT)
    s[OUT_FEAT:, :] = np.eye(OUT_FEAT)
    return s.astype(ml_dtypes.bfloat16)


def _block_order(c):
    """Column blocks for core c: partners c+1..c+4 (mod 8), own block last."""
    return [(c + 1 + s) % 8 for s in range(4)] + [c]


def _pack_pairs(a):
    """[512, n] -> [256, 2n]: infeat chunks (2q, 2q+1) interleaved in the
    free dim (the DoubleRow pair dimension)."""
    n = a.shape[1]
    out = np.empty((256, 2 * n), dtype=a.dtype)
    for q in range(2):
        out[q * 128:(q + 1) * 128, 0:n] = a[(2 * q) * 128:(2 * q + 1) * 128]
        out[q * 128:(q + 1) * 128, n:2 * n] = \
            a[(2 * q + 1) * 128:(2 * q + 2) * 128]
    return out


def _in_maps(x, T):
    f8 = ml_dtypes.float8_e4m3
    ax = _pow2_scale(float(np.abs(x).max()))
    aT = _pow2_scale(float(np.abs(T).max()))
    beta = 2.0 ** 12 / (ax * aT)
    T2 = T.reshape(IN_FEAT, OK)
    Tb = _pack_pairs((T2 * aT).astype(f8))
    Tsum = T2.reshape(IN_FEAT, OUT_FEAT, KERNEL_DIM)
    aTs = _pow2_scale(float(np.abs(Tsum.sum(axis=2)).max()))
    Tsb = _Ts_host(Tsum, aTs)
    betaS = 2.0 ** 12 / (aTs * ax)
    selb = _selb_host(beta)
    seldr = _seldr_host(beta, betaS)
    sel2b = _sel2_host()
    nbeta = np.full((OUT_FEAT, 1), -betaS, dtype=np.float32)
    xT = np.ascontiguousarray(x.T) * ax
    maps = []
    for c in range(N_CORES):
        xTc = np.empty((IN_FEAT, FD), dtype=np.float32)
        for pos, b in enumerate(_block_order(c)):
            xTc[:, 64 * pos:64 * (pos + 1)] = xT[:, 64 * b:64 * (b + 1)]
        maps.append({"xT": _pack_pairs(xTc.astype(f8)), "Tm": Tb,
                     "Ts": Tsb, "selb": selb, "seldr": seldr,
                     "sel2": sel2b, "nbeta": nbeta})
    return maps


def _assemble(x, results):
    """results: list of 8 dicts with 'rowS' [o, i] and 'accS' [o, 320]."""
    mbd = np.zeros((BATCH, OUT_FEAT), dtype=np.float32)
    for c in range(N_CORES):
        rs = np.concatenate([np.asarray(results[c]["rowsA"], np.float32),
                             np.asarray(results[c]["rowsB"], np.float32)],
                            axis=1)
        acc = sum(np.asarray(results[c][f"acc{ch}"], np.float32)
                  for ch in range(ROWB // CHUNK))
        # odd rows' diagonal self term (E_ii = 1) is missing from the
        # pair-packed column sums
        acc[:, DIAG0 + 1::2] += 1.0
        # own rows: row sums (j <= i) + diagonal transpose tail (j > i,
        # = accS diag col i minus the double-counted E_ii)
        mbd[64 * c:64 * (c + 1), :] += rs.T + acc[:, DIAG0:].T - 1.0
        # partner transposes: positions 0..2 (blocks c+1..c+3); position 3
        # (c+4) is the duplicated pair, covered by that core's own rowS
        for s in range(3):
            b = (c + 1 + s) % 8
            mbd[64 * b:64 * (b + 1), :] += acc[:, 64 * s:64 * (s + 1)].T
    mbd -= 1.0  # reference subtracts the self-similarity exp(0)=1
    return np.concatenate([np.asarray(x, np.float32), mbd], axis=1)


def kernel(x, T):
    from concourse import bass_utils

    x = np.asarray(x, dtype=np.float32)
    T = np.asarray(T, dtype=np.float32)

    if "nc" not in _cache:
        _cache["nc"] = _build_nc()
    nc = _cache["nc"]

    res = bass_utils.run_bass_kernel_spmd(
        nc, _in_maps(x, T), core_ids=list(range(N_CORES)))
    return _assemble(x, res.results)


# revision 32
# speedup vs baseline: 1.0309x; 1.0309x over previous
"""MiniBatchDiscrimination kernel, v5.1: fp8 DoubleRow everywhere it pays
+ triangle-trimmed diagonal block + 3-engine elementwise split.

Per core (row block c): columns = 4 partner blocks (c+1..c+4 mod 8) at
positions 0-3 and the OWN (diagonal) block last, cols 256:320.  Per row i
only cols [0, 257+i) are computed (j <= i within the diagonal block); the
j > i half comes from the column sums by symmetry (4 block pairs are
computed twice, once per orientation, so every core carries 5 real
blocks and no poison).

  M = x @ T on PE as fp8 DoubleRow matmuls: inputs are host-quantized
  x*ax, T*aT with ax, aT POWERS OF TWO, so the compensation beta =
  2^12/(ax aT) is exact in bf16 and fp8 and is folded into the selector
  weights at zero cost.  Mt'' tiles (bf16) = psum * 2^-12 (fixed scale).
  Per i:  d_t = Mt_t - Mt_t[:, 256+i] per tile:
    tiles 0-9   DVE  relu(d) -> bf16     (weight 2 beta, bf16 matmuls)
    tiles 10,11 ACT  |d| via Abs -> fp8  (weight beta, DR pair 0)
    tiles 12-14 Pool relu(d) -> fp8     (weight 2 beta, DR pairs 1-2)
    tile  15    DVE  relu(d) -> fp8     (weight 2 beta, DR pair 2)
  (the DVE/Pool TensorScalar ISA has no (subtract, abs_max), hence relu
  with |d| = 2 relu(d) - d and the S correction below; ACT's Abs is a
  real activation function.)
  Reduction on PE into one PSUM bank [64, W]: 10 bf16 matmuls + 3 fp8
  DoubleRow pair matmuls + 1 DR pair (-beta * S8, 0-weight dummy) that
  applies the -S_j correction.  S8 = fp8(sum_k Mt'' over relu tiles).
  The Exp bias +S_i is -nbeta*S8[:, 256+i] computed from the SAME fp8
  values, so D_ii == 0 exactly and exp(-0) = 1 lands in rowS/accS.
  Exp on ACT packs e tiles (2 rows each) with accum_out -> rowS.
  Column sums per 16-row chunk on PE: a zero-weight full-width matmul
  opens the PSUM accumulation group, then one pair matmul right after
  each odd row's Exp (ascending widths), accumulated into accS on DVE.

Host: mbd rows c = rowS_c + accS transposes from cores c-1..c-3 +
own-diagonal accS tail (+1 for odd rows' self term missing from the
pair-packed column sums, -1 for the double-counted E_ii, -1 for the
reference's self-similarity subtraction).
"""

import math

import numpy as np
import ml_dtypes
from contextlib import ExitStack

BATCH, IN_FEAT, OUT_FEAT, KERNEL_DIM = 512, 512, 64, 32
N_CORES = 8
ROWB = BATCH // N_CORES          # 64 rows of i per core
OK = OUT_FEAT * KERNEL_DIM       # 2048 flattened (o,k)
NT = OK // 128                   # 16 partition-tiles of (o,k)
NBLK = 5                         # column blocks per core
FD = NBLK * 64                   # 320
PBASE = 3 * 64                   # 192: start of the interleaved tail
# cols 0:192 = full partner blocks c+1..c+3; cols 192:320 interleave the
# HALF partner block c+4 (even slots 192+2m) with the diagonal block (odd
# slots 193+2m).  Row i computes cols [0, 194+2i): both tail blocks
# triangle-trimmed (j <= i), their j > i halves come from column sums by
# symmetry (the c+4 pair exists in both orientations across cores).

DVE_BF = (0, 1, 2, 3, 4, 5, 6, 7, 8, 9)
PAIRS = ((10, 11), (12, 13), (14, 15))
PROD = {10: "act", 11: "act", 12: "pool", 13: "pool", 14: "pool",
        15: "dve8"}
RELU_T = DVE_BF + tuple(t for t, e in PROD.items() if e != "act")
CHUNK = 16                       # i's per colsum chunk
SELW = OUT_FEAT
# iterations where tile 15 is produced on Pool instead of DVE (fractional
# load balance: DVE and Pool rates differ ~3.7x)
T15_POOL = frozenset(i for i in range(ROWB) if i % 8 == 3)
OUT_NAMES = ("rowsA", "rowsB", "acc0", "acc1", "acc2", "acc3")

_cache = {}


def _build_nc(split_waits=True):
    import concourse.bass as bass
    import concourse.mybir as mybir
    import concourse.tile as tile

    dt = mybir.dt
    AF = mybir.ActivationFunctionType
    OP = mybir.AluOpType
    DR = mybir.MatmulPerfMode.DoubleRow

    nc = bass.Bass("TRN2", target_bir_lowering=False, debug=False,
                   num_devices=N_CORES)

    # fp8 inputs for the DoubleRow M matmuls: pack q holds infeat chunks
    # (2q, 2q+1) interleaved in the free dim (the DR pair dimension)
    T_d = nc.dram_tensor("Tm", [2 * 128, 2 * OK], dt.float8e4,
                         kind="ExternalInput")
    xT_d = nc.dram_tensor("xT", [2 * 128, 2 * FD], dt.float8e4,
                          kind="ExternalInput")
    selb_d = nc.dram_tensor("selb", [128, len(DVE_BF) * SELW], dt.bfloat16,
                            kind="ExternalInput")
    seldr_d = nc.dram_tensor("seldr", [128, (len(PAIRS) + 1) * 128],
                             dt.float8e4, kind="ExternalInput")
    sel2_d = nc.dram_tensor("sel2", [128, OUT_FEAT], dt.bfloat16,
                            kind="ExternalInput")
    Ts_d = nc.dram_tensor("Ts", [2 * 128, 2 * SELW], dt.float8e4,
                          kind="ExternalInput")
    nbeta_d = nc.dram_tensor("nbeta", [OUT_FEAT, 1], dt.float32,
                             kind="ExternalInput")
    out_d = {}
    for nm in ("rowsA", "rowsB"):
        out_d[nm] = nc.dram_tensor(nm, [OUT_FEAT, ROWB // 2], dt.float32,
                                   kind="ExternalOutput")
    for ch in range(ROWB // CHUNK):
        out_d[f"acc{ch}"] = nc.dram_tensor(f"acc{ch}", [OUT_FEAT, FD],
                                           dt.float32,
                                           kind="ExternalOutput")

    with tile.TileContext(nc) as tc, ExitStack() as ctx:
        const = ctx.enter_context(tc.tile_pool(name="const", bufs=1))
        mtp = ctx.enter_context(tc.tile_pool(name="mt", bufs=NT))
        advp = ctx.enter_context(tc.tile_pool(name="adv", bufs=30))
        pairp = ctx.enter_context(tc.tile_pool(name="pair", bufs=9))
        ep = ctx.enter_context(tc.tile_pool(name="e", bufs=20))
        psda = ctx.enter_context(
            tc.tile_pool(name="psda", bufs=3, space=bass.MemorySpace.PSUM))
        psc = ctx.enter_context(
            tc.tile_pool(name="psc", bufs=2, space=bass.MemorySpace.PSUM))

        # ---- input DMAs: all on the SP HWDGE queue (cheapest setups, and
        # DMAs on the ACT queue would clog ACT's sequencer), first-needed
        # first.  Tm is packed per-tile-contiguous so each half unlocks 8
        # M tiles ----
        Tsb, xsb = [], []
        for q in range(2):
            t_ = const.tile([128, 2 * OK], dt.float8e4, tag=f"T{q}")
            Tsb.append(t_)
            x_ = const.tile([128, 2 * FD], dt.float8e4, tag=f"x{q}")
            xsb.append(x_)
        nc.sync.dma_start(xsb[0][:], xT_d[0:128, :])
        nc.sync.dma_start(xsb[1][:], xT_d[128:256, :])
        for h in range(2):
            for q in range(2):
                nc.sync.dma_start(Tsb[q][:, h * OK:(h + 1) * OK],
                                  T_d[q * 128:(q + 1) * 128,
                                      h * OK:(h + 1) * OK])
        Ts = []
        for q in range(2):
            ts_ = const.tile([128, 2 * SELW], dt.float8e4, tag=f"Ts{q}")
            nc.sync.dma_start(ts_[:], Ts_d[q * 128:(q + 1) * 128, :])
            Ts.append(ts_)
        nbeta = const.tile([OUT_FEAT, 1], dt.float32, tag="nbeta")
        nc.sync.dma_start(nbeta[:], nbeta_d[:])
        selb = const.tile([128, len(DVE_BF) * SELW], dt.bfloat16, tag="selb")
        nc.sync.dma_start(selb[:], selb_d[:])
        seldr = const.tile([128, (len(PAIRS) + 1) * 128], dt.float8e4,
                           tag="seldr")
        nc.sync.dma_start(seldr[:], seldr_d[:])
        sel2 = const.tile([128, OUT_FEAT], dt.bfloat16, tag="sel2")
        nc.sync.dma_start(sel2[:], sel2_d[:])

        rowS = const.tile([OUT_FEAT, ROWB], dt.float32, tag="rowS")
        accsb = []
        for ch in range(ROWB // CHUNK):
            a_ = const.tile([OUT_FEAT, FD], dt.float32, tag=f"accsb{ch}")
            accsb.append(a_)
        # fp32 image of the (rounded) bf16 diagonal columns: the subtracted
        # scalar exactly equals the tile value, so d_ii == 0 exactly
        mcol = const.tile([128, NT * ROWB], dt.float32, tag="mcol")
        # zero weights: opens each colsum PSUM group at full width
        zw = const.tile([128, SELW], dt.bfloat16, tag="zw")
        nc.vector.memset(zw[:], 0.0)

        # ---- M preamble: Mt''[(o,k), j] bf16 tiles via DR matmuls ----
        mts = []
        with tc.tile_pool(name="psm", bufs=3,
                          space=bass.MemorySpace.PSUM) as psm:
            # S via the precontracted Ts = sum_k T (relu tiles' o only):
            # needs only the small early DMAs, and warms the PE p-state
            psS = psc.tile([OUT_FEAT, FD], dt.float32, tag="psc",
                           name="psS")
            for q in range(2):
                nc.tensor.matmul(
                    psS[:],
                    Ts[q][:].rearrange("p (two m) -> p two m", two=2),
                    xsb[q][:].rearrange("p (two n) -> p two n", two=2),
                    start=(q == 0), stop=(q == 1), perf_mode=DR)
            S8 = const.tile([OUT_FEAT, 2 * FD], dt.float8e4, tag="S8")
            nc.scalar.activation(S8[:, 0:FD], psS[:], AF.Copy,
                                 scale=2.0 ** -12)
            nc.scalar.activation(S8[:, FD:2 * FD], psS[:], AF.Copy,
                                 scale=2.0 ** -12)
            Sneg = const.tile([OUT_FEAT, ROWB], dt.float32, tag="Sneg")
            nc.vector.tensor_scalar(
                Sneg[:],
                S8[:, PBASE:FD].rearrange("p (m two) -> p two m",
                                          two=2)[:, 1, :],
                nbeta[:], None, op0=OP.mult)
            for t in range(NT):
                ps = psm.tile([128, FD], dt.float32, tag="psm",
                              name=f"psm_{t}")
                for q in range(2):
                    nc.tensor.matmul(
                        ps[:],
                        Tsb[q][:, t * 256:(t + 1) * 256].rearrange(
                            "p (two m) -> p two m", two=2),
                        xsb[q][:].rearrange("p (two n) -> p two n", two=2),
                        start=(q == 0), stop=(q == 1), perf_mode=DR)
                mt_t = mtp.tile([128, FD], dt.bfloat16, tag="mt",
                                name=f"mt_{t}")
                # descale by the FIXED 2^-12 (the data-dependent remainder
                # beta lives in the selector weights); parity-interleaved
                # so both copy chains start with the first Tm half
                if t % 2 == 0:
                    nc.vector.tensor_scalar(mt_t[:], ps[:], 2.0 ** -12,
                                            None, op0=OP.mult)
                else:
                    nc.scalar.activation(mt_t[:], ps[:], AF.Copy,
                                         scale=2.0 ** -12)
                nc.gpsimd.tensor_copy(
                    mcol[:, t * ROWB:(t + 1) * ROWB],
                    mt_t[:, PBASE:FD].rearrange(
                        "p (m two) -> p two m", two=2)[:, 1, :])
                mts.append(mt_t)

        e_lists = [[] for _ in range(ROWB // CHUNK)]
        pend_exp = None
        pc_cur = [None]

        def emit_exp(i, ps_i):
            W = PBASE + 2 * i + 2
            ch = i // CHUNK
            if i % 2 == 0:
                e_t = ep.tile([128, FD], dt.bfloat16, tag="e",
                              name=f"e_{i}")
                e_lists[ch].append(e_t)
            half = e_lists[ch][-1][(i % 2) * OUT_FEAT:
                                   (i % 2 + 1) * OUT_FEAT, 0:W]
            nc.scalar.activation(half, ps_i[:, 0:W], AF.Exp,
                                 scale=-1.0, bias=Sneg[:, i:i + 1],
                                 accum_out=rowS[:, i:i + 1])
            if i == ROWB // 2 - 1:
                nc.sync.dma_start(out_d["rowsA"][:],
                                  rowS[:, 0:ROWB // 2])
            elif i == ROWB - 1:
                nc.sync.dma_start(out_d["rowsB"][:],
                                  rowS[:, ROWB // 2:ROWB])
            if i % 2 == 1:
                # colsum pair matmul for rows (i-1, i), ascending widths;
                # a zero-weight full-width matmul opened the group so the
                # PSUM zero region is already cleared.  The strict width
                # (192+2*(i-1)) excludes self and pair-diagonal terms; the
                # 2-col fixup adds the odd row's last two entries
                m = (i % CHUNK) // 2
                ie = i - 1
                if m == 0:
                    pc_cur[0] = psc.tile([OUT_FEAT, FD], dt.float32,
                                         tag="psc", name=f"psc_{ch}")
                    nc.tensor.matmul(pc_cur[0][:], zw[:, 0:SELW],
                                     mts[0][:], start=True, stop=False)
                Wp = PBASE + 2 * ie
                nc.tensor.matmul(pc_cur[0][:, 0:Wp], sel2[:],
                                 e_lists[ch][m][:, 0:Wp],
                                 start=False, stop=False)
                nc.tensor.matmul(pc_cur[0][:, Wp:Wp + 2],
                                 sel2[OUT_FEAT:128, :],
                                 e_lists[ch][m][OUT_FEAT:128, Wp:Wp + 2],
                                 start=False, stop=(m == CHUNK // 2 - 1))
                if m == CHUNK // 2 - 1:
                    # stage the chunk's column sums to SBUF and DMA out on
                    # the idle SP queue; the zero-weight opener zeroed the
                    # tail columns.  Last chunk copies on DVE (idle at the
                    # tail, and ACT still owes the final Exp)
                    if ch == ROWB // CHUNK - 1:
                        nc.vector.tensor_copy(accsb[ch][:], pc_cur[0][:])
                    else:
                        nc.scalar.activation(accsb[ch][:], pc_cur[0][:],
                                             AF.Copy)
                    nc.sync.dma_start(out_d[f"acc{ch}"][:], accsb[ch][:])

        for i in range(ROWB):
            W = PBASE + 2 * i + 2
            ps_i = psda.tile([OUT_FEAT, FD], dt.float32, tag="psda",
                             name=f"psda_{i}")
            # ---- elementwise tiles ----
            ads = {}
            pbs = []
            for pi, (ta, tb) in enumerate(PAIRS):
                pb = pairp.tile([128, 2 * FD], dt.float8e4, tag="pb",
                                name=f"pb_{i}_{pi}")
                pbs.append(pb)
                ads[ta] = pb[:, 0:W]
                ads[tb] = pb[:, FD:FD + W]
            # DVE fp8 half first so its consumer (last DR matmul) is ready
            for t, eng in PROD.items():
                if eng != "dve8":
                    continue
                sc = mcol[:, t * ROWB + i:t * ROWB + i + 1]
                e_ = nc.gpsimd if i in T15_POOL else nc.vector
                e_.tensor_scalar(ads[t], mts[t][:, 0:W], sc, 0.0,
                                 op0=OP.subtract, op1=OP.max)
            for t in DVE_BF:
                sc = mcol[:, t * ROWB + i:t * ROWB + i + 1]
                ad_t = advp.tile([128, FD], dt.bfloat16, tag="adv",
                                 name=f"ad_{i}_{t}")
                nc.vector.tensor_scalar(ad_t[:, 0:W], mts[t][:, 0:W], sc,
                                        0.0, op0=OP.subtract, op1=OP.max)
                ads[t] = ad_t[:, 0:W]
            for t, eng in PROD.items():
                sc = mcol[:, t * ROWB + i:t * ROWB + i + 1]
                if eng == "act":
                    nc.scalar.activation(ads[t], mts[t][:, 0:W], AF.Abs,
                                         bias=sc, scale=-1.0)
                elif eng == "pool":
                    nc.gpsimd.tensor_scalar(ads[t], mts[t][:, 0:W], sc, 0.0,
                                            op0=OP.subtract, op1=OP.max)
            # ---- reduction matmuls into one PSUM bank ----
            for m, t in enumerate(DVE_BF):
                nc.tensor.matmul(ps_i[:, 0:W],
                                 selb[:, m * SELW:(m + 1) * SELW],
                                 ads[t], start=(m == 0), stop=False)
            for pi in range(len(PAIRS)):
                nc.tensor.matmul(
                    ps_i[:, 0:W],
                    seldr[:, pi * 128:(pi + 1) * 128].rearrange(
                        "p (two m) -> p two m", two=2),
                    pbs[pi][:, 0:2 * FD].rearrange(
                        "p (two n) -> p two n", two=2)[:, :, 0:W],
                    start=False, stop=False,
                    perf_mode=DR)
            # -S_j correction: DR pair (-beta * S8, zero-weight dummy)
            npi = len(PAIRS)
            nc.tensor.matmul(
                ps_i[:, 0:W],
                seldr[0:OUT_FEAT,
                      npi * 128:(npi + 1) * 128].rearrange(
                    "p (two m) -> p two m", two=2),
                S8[:, 0:2 * FD].rearrange(
                    "p (two n) -> p two n", two=2)[:, :, 0:W],
                start=False, stop=True, perf_mode=DR)
            # ---- software-pipelined Exp (+ inline colsum pairs) ----
            if pend_exp is not None:
                emit_exp(*pend_exp)
            pend_exp = (i, ps_i)
        emit_exp(*pend_exp)

    if split_waits:
        _split_multiwaits(nc, mybir)
    return nc


def _split_multiwaits(nc, mybir):
    """Walrus encodes at most ONE sync-wait command per instruction. Split
    any instruction with more into a chain of single-wait Drain carriers on
    the same engine, inserted immediately before it."""
    n = 0
    for fn in nc.m.functions:
        for bb in fn.blocks:
            new_insts = []
            for inst in bb.instructions:
                si = getattr(inst, "sync_info", None)
                if si is not None and si.on_wait and len(si.on_wait) > 1:
                    waits = list(si.on_wait)
                    for w in waits[:-1]:
                        carrier = mybir.InstDrain(
                            name=f"splitw_{n}", engine=inst.engine,
                            ins=[], outs=[],
                            sync_info=mybir.SyncInfo(on_wait=[w],
                                                     on_update=[]))
                        new_insts.append(carrier)
                        n += 1
                    inst.sync_info = mybir.SyncInfo(
                        on_wait=[waits[-1]], on_update=list(si.on_update))
                new_insts.append(inst)
            if n:
                bb.instructions = new_insts


def _pow2_scale(m, target=200.0):
    """Largest power of two a with m * a <= target."""
    if not np.isfinite(m) or m <= 0:
        return 1.0
    return 2.0 ** math.floor(math.log2(target / m))


def _selb_host(beta):
    sel = np.zeros((128, len(DVE_BF) * SELW), dtype=np.float32)
    for m, t in enumerate(DVE_BF):
        for g in range(4):
            sel[32 * g:32 * (g + 1), m * SELW + 4 * t + g] = 2.0 * beta
    return sel.astype(ml_dtypes.bfloat16)


def _seldr_host(beta, betaS):
    sel = np.zeros((128, (len(PAIRS) + 1) * 128), dtype=np.float32)
    for pi, pair in enumerate(PAIRS):
        for h, t in enumerate(pair):
            v = beta if PROD[t] == "act" else 2.0 * beta
            for g in range(4):
                sel[32 * g:32 * (g + 1),
                    pi * 128 + h * SELW + 4 * t + g] = v
    # pair 3: half0 = -betaS * I64 (the S correction), half1 = 0
    npi = len(PAIRS)
    sel[0:OUT_FEAT, npi * 128:npi * 128 + OUT_FEAT] = \
        -betaS * np.eye(OUT_FEAT, dtype=np.float32)
    return sel.astype(ml_dtypes.float8_e4m3)


def _Ts_host(T, aTs):
    """Precontracted T for the S matmuls: Ts[i, o] = sum_k T[i, o, k] for
    o's whose tile holds relu values, else 0; packed for DoubleRow."""
    Ts = T.sum(axis=2) * aTs                      # [IN_FEAT, OUT_FEAT]
    mask = np.zeros(OUT_FEAT, dtype=np.float32)
    for t in RELU_T:
        mask[4 * t:4 * t + 4] = 1.0
    return _pack_pairs((Ts * mask).astype(ml_dtypes.float8_e4m3))


def _sel2_host():
    s = np.zeros((128, OUT_FEAT), dtype=np.float32)
    s[:OUT_FEAT, :] = np.eye(OUT_FEAT)
    s[OUT_FEAT:, :] = np.eye(OUT_FEAT)
    return s.astype(ml_dtypes.bfloat16)


def _block_order(c):
    """Column blocks for core c: partners c+1..c+4 (mod 8), own block last."""
    return [(c + 1 + s) % 8 for s in range(4)] + [c]


def _pack_pairs(a):
    """[512, n] -> [256, 2n]: infeat chunks (2q, 2q+1) interleaved in the
    free dim (the DoubleRow pair dimension)."""
    n = a.shape[1]
    out = np.empty((256, 2 * n), dtype=a.dtype)
    for q in range(2):
        out[q * 128:(q + 1) * 128, 0:n] = a[(2 * q) * 128:(2 * q + 1) * 128]
        out[q * 128:(q + 1) * 128, n:2 * n] = \
            a[(2 * q + 1) * 128:(2 * q + 2) * 128]
    return out


def _pack_pairs_tiled(a):
    """Like _pack_pairs but tile-contiguous: tile t (128 cols) occupies
    cols [t*256, (t+1)*256) with its pair halves side by side, so half
    the Tm DMA unlocks 8 M tiles."""
    n = a.shape[1]
    nt = n // 128
    out = np.empty((256, 2 * n), dtype=a.dtype)
    for q in range(2):
        for t in range(nt):
            out[q * 128:(q + 1) * 128, t * 256:t * 256 + 128] = \
                a[(2 * q) * 128:(2 * q + 1) * 128, t * 128:(t + 1) * 128]
            out[q * 128:(q + 1) * 128, t * 256 + 128:(t + 1) * 256] = \
                a[(2 * q + 1) * 128:(2 * q + 2) * 128,
                  t * 128:(t + 1) * 128]
    return out


def _in_maps(x, T):
    f8 = ml_dtypes.float8_e4m3
    ax = _pow2_scale(float(np.abs(x).max()))
    aT = _pow2_scale(float(np.abs(T).max()))
    beta = 2.0 ** 12 / (ax * aT)
    T2 = T.reshape(IN_FEAT, OK)
    Tb = _pack_pairs_tiled((T2 * aT).astype(f8))
    Tsum = T2.reshape(IN_FEAT, OUT_FEAT, KERNEL_DIM)
    aTs = _pow2_scale(float(np.abs(Tsum.sum(axis=2)).max()))
    Tsb = _Ts_host(Tsum, aTs)
    betaS = 2.0 ** 12 / (aTs * ax)
    selb = _selb_host(beta)
    seldr = _seldr_host(beta, betaS)
    sel2b = _sel2_host()
    nbeta = np.full((OUT_FEAT, 1), -betaS, dtype=np.float32)
    xT = np.ascontiguousarray(x.T) * ax
    maps = []
    for c in range(N_CORES):
        xTc = np.empty((IN_FEAT, FD), dtype=np.float32)
        for pos, b in enumerate(_block_order(c)):
            xTc[:, 64 * pos:64 * (pos + 1)] = xT[:, 64 * b:64 * (b + 1)]
        maps.append({"xT": _pack_pairs(xTc.astype(f8)), "Tm": Tb,
                     "Ts": Tsb, "selb": selb, "seldr": seldr,
                     "sel2": sel2b, "nbeta": nbeta})
    return maps


def _assemble(x, results):
    """results: list of 8 dicts with 'rowS' [o, i] and 'accS' [o, 320]."""
    mbd = np.zeros((BATCH, OUT_FEAT), dtype=np.float32)
    for c in range(N_CORES):
        rs = np.concatenate([np.asarray(results[c]["rowsA"], np.float32),
                             np.asarray(results[c]["rowsB"], np.float32)],
                            axis=1)
        acc = sum(np.asarray(results[c][f"acc{ch}"], np.float32)
                  for ch in range(ROWB // CHUNK))
        # odd rows' diagonal self term (E_ii = 1) is missing from the
        # pair-packed column sums
        acc[:, DIAG0 + 1::2] += 1.0
        # own rows: row sums (j <= i) + diagonal transpose tail (j > i,
        # = accS diag col i minus the double-counted E_ii)
        mbd[64 * c:64 * (c + 1), :] += rs.T + acc[:, DIAG0:].T - 1.0
        # partner transposes: positions 0..2 (blocks c+1..c+3); position 3
        # (c+4) is the duplicated pair, covered by that core's own rowS
        for s in range(3):
            b = (c + 1 + s) % 8
            mbd[64 * b:64 * (b + 1), :] += acc[:, 64 * s:64 * (s + 1)].T
    mbd -= 1.0  # reference subtracts the self-similarity exp(0)=1
    return np.concatenate([np.asarray(x, np.float32), mbd], axis=1)


def kernel(x, T):
    from concourse import bass_utils

    x = np.asarray(x, dtype=np.float32)
    T = np.asarray(T, dtype=np.float32)

    if "nc" not in _cache:
        _cache["nc"] = _build_nc()
    nc = _cache["nc"]

    res = bass_utils.run_bass_kernel_spmd(
        nc, _in_maps(x, T), core_ids=list(range(N_CORES)))
    return _assemble(x, res.results)


# revision 33
# speedup vs baseline: 1.1039x; 1.0708x over previous
"""MiniBatchDiscrimination kernel, v5.1: fp8 DoubleRow everywhere it pays
+ triangle-trimmed diagonal block + 3-engine elementwise split.

Per core (row block c): columns = 4 partner blocks (c+1..c+4 mod 8) at
positions 0-3 and the OWN (diagonal) block last, cols 256:320.  Per row i
only cols [0, 257+i) are computed (j <= i within the diagonal block); the
j > i half comes from the column sums by symmetry (4 block pairs are
computed twice, once per orientation, so every core carries 5 real
blocks and no poison).

  M = x @ T on PE as fp8 DoubleRow matmuls: inputs are host-quantized
  x*ax, T*aT with ax, aT POWERS OF TWO, so the compensation beta =
  2^12/(ax aT) is exact in bf16 and fp8 and is folded into the selector
  weights at zero cost.  Mt'' tiles (bf16) = psum * 2^-12 (fixed scale).
  Per i:  d_t = Mt_t - Mt_t[:, 256+i] per tile:
    tiles 0-9   DVE  relu(d) -> bf16     (weight 2 beta, bf16 matmuls)
    tiles 10,11 ACT  |d| via Abs -> fp8  (weight beta, DR pair 0)
    tiles 12-14 Pool relu(d) -> fp8     (weight 2 beta, DR pairs 1-2)
    tile  15    DVE  relu(d) -> fp8     (weight 2 beta, DR pair 2)
  (the DVE/Pool TensorScalar ISA has no (subtract, abs_max), hence relu
  with |d| = 2 relu(d) - d and the S correction below; ACT's Abs is a
  real activation function.)
  Reduction on PE into one PSUM bank [64, W]: 10 bf16 matmuls + 3 fp8
  DoubleRow pair matmuls + 1 DR pair (-beta * S8, 0-weight dummy) that
  applies the -S_j correction.  S8 = fp8(sum_k Mt'' over relu tiles).
  The Exp bias +S_i is -nbeta*S8[:, 256+i] computed from the SAME fp8
  values, so D_ii == 0 exactly and exp(-0) = 1 lands in rowS/accS.
  Exp on ACT packs e tiles (2 rows each) with accum_out -> rowS.
  Column sums per 16-row chunk on PE: a zero-weight full-width matmul
  opens the PSUM accumulation group, then one pair matmul right after
  each odd row's Exp (ascending widths), accumulated into accS on DVE.

Host: mbd rows c = rowS_c + accS transposes from cores c-1..c-3 +
own-diagonal accS tail (+1 for odd rows' self term missing from the
pair-packed column sums, -1 for the double-counted E_ii, -1 for the
reference's self-similarity subtraction).
"""

import math

import numpy as np
import ml_dtypes
from contextlib import ExitStack

BATCH, IN_FEAT, OUT_FEAT, KERNEL_DIM = 512, 512, 64, 32
N_CORES = 8
ROWB = BATCH // N_CORES          # 64 rows of i per core
OK = OUT_FEAT * KERNEL_DIM       # 2048 flattened (o,k)
NT = OK // 128                   # 16 partition-tiles of (o,k)
NBLK = 5                         # column blocks per core
FD = NBLK * 64                   # 320
PBASE = 3 * 64                   # 192: start of the interleaved tail
# cols 0:192 = full partner blocks c+1..c+3; cols 192:320 interleave the
# HALF partner block c+4 (even slots 192+2m) with the diagonal block (odd
# slots 193+2m).  Row i computes cols [0, 194+2i): both tail blocks
# triangle-trimmed (j <= i), their j > i halves come from column sums by
# symmetry (the c+4 pair exists in both orientations across cores).

DVE_BF = (0, 1, 2, 3, 4, 5, 6, 7, 8, 9)
PAIRS = ((10, 11), (12, 13), (14, 15))
PROD = {10: "act", 11: "act", 12: "pool", 13: "pool", 14: "pool",
        15: "dve8"}
RELU_T = DVE_BF + tuple(t for t, e in PROD.items() if e != "act")
CHUNK = 16                       # i's per colsum chunk
SELW = OUT_FEAT
# iterations where tile 15 is produced on Pool instead of DVE (fractional
# load balance: DVE and Pool rates differ ~3.7x)
T15_POOL = frozenset(i for i in range(ROWB) if i % 8 == 3)
OUT_NAMES = ("rowsA", "rowsB", "acc0", "acc1", "acc2", "acc3")

_cache = {}


def _build_nc(split_waits=True):
    import concourse.bass as bass
    import concourse.mybir as mybir
    import concourse.tile as tile

    dt = mybir.dt
    AF = mybir.ActivationFunctionType
    OP = mybir.AluOpType
    DR = mybir.MatmulPerfMode.DoubleRow

    nc = bass.Bass("TRN2", target_bir_lowering=False, debug=False,
                   num_devices=N_CORES)

    # fp8 inputs for the DoubleRow M matmuls: pack q holds infeat chunks
    # (2q, 2q+1) interleaved in the free dim (the DR pair dimension)
    T_d = nc.dram_tensor("Tm", [2 * 128, 2 * OK], dt.float8e4,
                         kind="ExternalInput")
    xT_d = nc.dram_tensor("xT", [2 * 128, 2 * FD], dt.float8e4,
                          kind="ExternalInput")
    selb_d = nc.dram_tensor("selb", [128, len(DVE_BF) * SELW], dt.bfloat16,
                            kind="ExternalInput")
    seldr_d = nc.dram_tensor("seldr", [128, (len(PAIRS) + 1) * 128],
                             dt.float8e4, kind="ExternalInput")
    sel2_d = nc.dram_tensor("sel2", [128, OUT_FEAT], dt.bfloat16,
                            kind="ExternalInput")
    Ts_d = nc.dram_tensor("Ts", [2 * 128, 2 * SELW], dt.float8e4,
                          kind="ExternalInput")
    nbeta_d = nc.dram_tensor("nbeta", [OUT_FEAT, 1], dt.float32,
                             kind="ExternalInput")
    out_d = {}
    for nm in ("rowsA", "rowsB"):
        out_d[nm] = nc.dram_tensor(nm, [OUT_FEAT, ROWB // 2], dt.float32,
                                   kind="ExternalOutput")
    for ch in range(ROWB // CHUNK):
        out_d[f"acc{ch}"] = nc.dram_tensor(f"acc{ch}", [OUT_FEAT, FD],
                                           dt.float32,
                                           kind="ExternalOutput")

    with tile.TileContext(nc) as tc, ExitStack() as ctx:
        const = ctx.enter_context(tc.tile_pool(name="const", bufs=1))
        mtp = ctx.enter_context(tc.tile_pool(name="mt", bufs=NT))
        advp = ctx.enter_context(tc.tile_pool(name="adv", bufs=30))
        pairp = ctx.enter_context(tc.tile_pool(name="pair", bufs=9))
        ep = ctx.enter_context(tc.tile_pool(name="e", bufs=20))
        psda = ctx.enter_context(
            tc.tile_pool(name="psda", bufs=3, space=bass.MemorySpace.PSUM))
        psc = ctx.enter_context(
            tc.tile_pool(name="psc", bufs=2, space=bass.MemorySpace.PSUM))

        # ---- input DMAs: all on the SP HWDGE queue (cheapest setups, and
        # DMAs on the ACT queue would clog ACT's sequencer), first-needed
        # first.  Tm is packed per-tile-contiguous so each half unlocks 8
        # M tiles ----
        Tsb, xsb = [], []
        for q in range(2):
            t_ = const.tile([128, 2 * OK], dt.float8e4, tag=f"T{q}")
            Tsb.append(t_)
            x_ = const.tile([128, 2 * FD], dt.float8e4, tag=f"x{q}")
            xsb.append(x_)
        nc.sync.dma_start(xsb[0][:], xT_d[0:128, :])
        nc.sync.dma_start(xsb[1][:], xT_d[128:256, :])
        for h in range(2):
            for q in range(2):
                nc.sync.dma_start(Tsb[q][:, h * OK:(h + 1) * OK],
                                  T_d[q * 128:(q + 1) * 128,
                                      h * OK:(h + 1) * OK])
        Ts = []
        for q in range(2):
            ts_ = const.tile([128, 2 * SELW], dt.float8e4, tag=f"Ts{q}")
            nc.sync.dma_start(ts_[:], Ts_d[q * 128:(q + 1) * 128, :])
            Ts.append(ts_)
        nbeta = const.tile([OUT_FEAT, 1], dt.float32, tag="nbeta")
        nc.sync.dma_start(nbeta[:], nbeta_d[:])
        selb = const.tile([128, len(DVE_BF) * SELW], dt.bfloat16, tag="selb")
        nc.sync.dma_start(selb[:], selb_d[:])
        seldr = const.tile([128, (len(PAIRS) + 1) * 128], dt.float8e4,
                           tag="seldr")
        nc.sync.dma_start(seldr[:], seldr_d[:])
        sel2 = const.tile([128, OUT_FEAT], dt.bfloat16, tag="sel2")
        nc.sync.dma_start(sel2[:], sel2_d[:])

        rowS = const.tile([OUT_FEAT, ROWB], dt.float32, tag="rowS")
        accsb = []
        for ch in range(ROWB // CHUNK):
            a_ = const.tile([OUT_FEAT, FD], dt.float32, tag=f"accsb{ch}")
            accsb.append(a_)
        # fp32 image of the (rounded) bf16 diagonal columns: the subtracted
        # scalar exactly equals the tile value, so d_ii == 0 exactly
        mcol = const.tile([128, NT * ROWB], dt.float32, tag="mcol")
        # zero weights: opens each colsum PSUM group at full width
        zw = const.tile([128, SELW], dt.bfloat16, tag="zw")
        nc.vector.memset(zw[:], 0.0)

        # ---- M preamble: Mt''[(o,k), j] bf16 tiles via DR matmuls ----
        mts = []
        with tc.tile_pool(name="psm", bufs=3,
                          space=bass.MemorySpace.PSUM) as psm:
            # S via the precontracted Ts = sum_k T (relu tiles' o only):
            # needs only the small early DMAs, and warms the PE p-state
            psS = psc.tile([OUT_FEAT, FD], dt.float32, tag="psc",
                           name="psS")
            for q in range(2):
                nc.tensor.matmul(
                    psS[:],
                    Ts[q][:].rearrange("p (two m) -> p two m", two=2),
                    xsb[q][:].rearrange("p (two n) -> p two n", two=2),
                    start=(q == 0), stop=(q == 1), perf_mode=DR)
            S8 = const.tile([OUT_FEAT, 2 * FD], dt.float8e4, tag="S8")
            nc.scalar.activation(S8[:, 0:FD], psS[:], AF.Copy,
                                 scale=2.0 ** -12)
            nc.scalar.activation(S8[:, FD:2 * FD], psS[:], AF.Copy,
                                 scale=2.0 ** -12)
            Sneg = const.tile([OUT_FEAT, ROWB], dt.float32, tag="Sneg")
            nc.vector.tensor_scalar(
                Sneg[:],
                S8[:, PBASE:FD].rearrange("p (m two) -> p two m",
                                          two=2)[:, 1, :],
                nbeta[:], None, op0=OP.mult)
            for t in range(NT):
                ps = psm.tile([128, FD], dt.float32, tag="psm",
                              name=f"psm_{t}")
                for q in range(2):
                    nc.tensor.matmul(
                        ps[:],
                        Tsb[q][:, t * 256:(t + 1) * 256].rearrange(
                            "p (two m) -> p two m", two=2),
                        xsb[q][:].rearrange("p (two n) -> p two n", two=2),
                        start=(q == 0), stop=(q == 1), perf_mode=DR)
                mt_t = mtp.tile([128, FD], dt.bfloat16, tag="mt",
                                name=f"mt_{t}")
                # descale by the FIXED 2^-12 (the data-dependent remainder
                # beta lives in the selector weights); parity-interleaved
                # so both copy chains start with the first Tm half
                if t % 2 == 0:
                    nc.vector.tensor_scalar(mt_t[:], ps[:], 2.0 ** -12,
                                            None, op0=OP.mult)
                else:
                    nc.scalar.activation(mt_t[:], ps[:], AF.Copy,
                                         scale=2.0 ** -12)
                nc.gpsimd.tensor_copy(
                    mcol[:, t * ROWB:(t + 1) * ROWB],
                    mt_t[:, PBASE:FD].rearrange(
                        "p (m two) -> p two m", two=2)[:, 1, :])
                mts.append(mt_t)

        e_lists = [[] for _ in range(ROWB // CHUNK)]
        pend_exp = None
        pc_cur = [None]

        def emit_exp(i, ps_i):
            W = PBASE + 2 * i + 2
            ch = i // CHUNK
            if i % 2 == 0:
                e_t = ep.tile([128, FD], dt.bfloat16, tag="e",
                              name=f"e_{i}")
                e_lists[ch].append(e_t)
            half = e_lists[ch][-1][(i % 2) * OUT_FEAT:
                                   (i % 2 + 1) * OUT_FEAT, 0:W]
            nc.scalar.activation(half, ps_i[:, 0:W], AF.Exp,
                                 scale=-1.0, bias=Sneg[:, i:i + 1],
                                 accum_out=rowS[:, i:i + 1])
            if i == ROWB // 2 - 1:
                nc.sync.dma_start(out_d["rowsA"][:],
                                  rowS[:, 0:ROWB // 2])
            elif i == ROWB - 1:
                nc.sync.dma_start(out_d["rowsB"][:],
                                  rowS[:, ROWB // 2:ROWB])
            if i % 2 == 1:
                # colsum pair matmul for rows (i-1, i), ascending widths;
                # a zero-weight full-width matmul opened the group so the
                # PSUM zero region is already cleared.  The strict width
                # (192+2*(i-1)) excludes self and pair-diagonal terms; the
                # 2-col fixup adds the odd row's last two entries
                m = (i % CHUNK) // 2
                ie = i - 1
                if m == 0:
                    pc_cur[0] = psc.tile([OUT_FEAT, FD], dt.float32,
                                         tag="psc", name=f"psc_{ch}")
                    nc.tensor.matmul(pc_cur[0][:], zw[:, 0:SELW],
                                     mts[0][:], start=True, stop=False)
                Wp = PBASE + 2 * ie
                nc.tensor.matmul(pc_cur[0][:, 0:Wp], sel2[:],
                                 e_lists[ch][m][:, 0:Wp],
                                 start=False, stop=False)
                nc.tensor.matmul(pc_cur[0][:, Wp:Wp + 2],
                                 sel2[OUT_FEAT:128, :],
                                 e_lists[ch][m][OUT_FEAT:128, Wp:Wp + 2],
                                 start=False, stop=(m == CHUNK // 2 - 1))
                if m == CHUNK // 2 - 1:
                    # stage the chunk's column sums to SBUF and DMA out on
                    # the idle SP queue; the zero-weight opener zeroed the
                    # tail columns.  Last chunk copies on DVE (idle at the
                    # tail, and ACT still owes the final Exp)
                    if ch == ROWB // CHUNK - 1:
                        nc.vector.tensor_copy(accsb[ch][:], pc_cur[0][:])
                    else:
                        nc.scalar.activation(accsb[ch][:], pc_cur[0][:],
                                             AF.Copy)
                    nc.sync.dma_start(out_d[f"acc{ch}"][:], accsb[ch][:])

        for i in range(ROWB):
            W = PBASE + 2 * i + 2
            ps_i = psda.tile([OUT_FEAT, FD], dt.float32, tag="psda",
                             name=f"psda_{i}")
            # ---- elementwise tiles ----
            ads = {}
            pbs = []
            for pi, (ta, tb) in enumerate(PAIRS):
                pb = pairp.tile([128, 2 * FD], dt.float8e4, tag="pb",
                                name=f"pb_{i}_{pi}")
                pbs.append(pb)
                ads[ta] = pb[:, 0:W]
                ads[tb] = pb[:, FD:FD + W]
            # DVE fp8 half first so its consumer (last DR matmul) is ready
            for t, eng in PROD.items():
                if eng != "dve8":
                    continue
                sc = mcol[:, t * ROWB + i:t * ROWB + i + 1]
                e_ = nc.gpsimd if i in T15_POOL else nc.vector
                e_.tensor_scalar(ads[t], mts[t][:, 0:W], sc, 0.0,
                                 op0=OP.subtract, op1=OP.max)
            for t in DVE_BF:
                sc = mcol[:, t * ROWB + i:t * ROWB + i + 1]
                ad_t = advp.tile([128, FD], dt.bfloat16, tag="adv",
                                 name=f"ad_{i}_{t}")
                nc.vector.tensor_scalar(ad_t[:, 0:W], mts[t][:, 0:W], sc,
                                        0.0, op0=OP.subtract, op1=OP.max)
                ads[t] = ad_t[:, 0:W]
            for t, eng in PROD.items():
                sc = mcol[:, t * ROWB + i:t * ROWB + i + 1]
                if eng == "act":
                    nc.scalar.activation(ads[t], mts[t][:, 0:W], AF.Abs,
                                         bias=sc, scale=-1.0)
                elif eng == "pool":
                    nc.gpsimd.tensor_scalar(ads[t], mts[t][:, 0:W], sc, 0.0,
                                            op0=OP.subtract, op1=OP.max)
            # ---- reduction matmuls into one PSUM bank ----
            for m, t in enumerate(DVE_BF):
                nc.tensor.matmul(ps_i[:, 0:W],
                                 selb[:, m * SELW:(m + 1) * SELW],
                                 ads[t], start=(m == 0), stop=False)
            for pi in range(len(PAIRS)):
                nc.tensor.matmul(
                    ps_i[:, 0:W],
                    seldr[:, pi * 128:(pi + 1) * 128].rearrange(
                        "p (two m) -> p two m", two=2),
                    pbs[pi][:, 0:2 * FD].rearrange(
                        "p (two n) -> p two n", two=2)[:, :, 0:W],
                    start=False, stop=False,
                    perf_mode=DR)
            # -S_j correction: DR pair (-beta * S8, zero-weight dummy)
            npi = len(PAIRS)
            nc.tensor.matmul(
                ps_i[:, 0:W],
                seldr[0:OUT_FEAT,
                      npi * 128:(npi + 1) * 128].rearrange(
                    "p (two m) -> p two m", two=2),
                S8[:, 0:2 * FD].rearrange(
                    "p (two n) -> p two n", two=2)[:, :, 0:W],
                start=False, stop=True, perf_mode=DR)
            # ---- software-pipelined Exp (+ inline colsum pairs) ----
            if pend_exp is not None:
                emit_exp(*pend_exp)
            pend_exp = (i, ps_i)
        emit_exp(*pend_exp)

    if split_waits:
        _split_multiwaits(nc, mybir)
    return nc


def _split_multiwaits(nc, mybir):
    """Walrus encodes at most ONE sync-wait command per instruction. Split
    any instruction with more into a chain of single-wait Drain carriers on
    the same engine, inserted immediately before it."""
    n = 0
    for fn in nc.m.functions:
        for bb in fn.blocks:
            new_insts = []
            for inst in bb.instructions:
                si = getattr(inst, "sync_info", None)
                if si is not None and si.on_wait and len(si.on_wait) > 1:
                    waits = list(si.on_wait)
                    for w in waits[:-1]:
                        carrier = mybir.InstDrain(
                            name=f"splitw_{n}", engine=inst.engine,
                            ins=[], outs=[],
                            sync_info=mybir.SyncInfo(on_wait=[w],
                                                     on_update=[]))
                        new_insts.append(carrier)
                        n += 1
                    inst.sync_info = mybir.SyncInfo(
                        on_wait=[waits[-1]], on_update=list(si.on_update))
                new_insts.append(inst)
            if n:
                bb.instructions = new_insts


def _pow2_scale(m, target=200.0):
    """Largest power of two a with m * a <= target."""
    if not np.isfinite(m) or m <= 0:
        return 1.0
    return 2.0 ** math.floor(math.log2(target / m))


def _selb_host(beta):
    sel = np.zeros((128, len(DVE_BF) * SELW), dtype=np.float32)
    for m, t in enumerate(DVE_BF):
        for g in range(4):
            sel[32 * g:32 * (g + 1), m * SELW + 4 * t + g] = 2.0 * beta
    return sel.astype(ml_dtypes.bfloat16)


def _seldr_host(beta, betaS):
    sel = np.zeros((128, (len(PAIRS) + 1) * 128), dtype=np.float32)
    for pi, pair in enumerate(PAIRS):
        for h, t in enumerate(pair):
            v = beta if PROD[t] == "act" else 2.0 * beta
            for g in range(4):
                sel[32 * g:32 * (g + 1),
                    pi * 128 + h * SELW + 4 * t + g] = v
    # pair 3: half0 = -betaS * I64 (the S correction), half1 = 0
    npi = len(PAIRS)
    sel[0:OUT_FEAT, npi * 128:npi * 128 + OUT_FEAT] = \
        -betaS * np.eye(OUT_FEAT, dtype=np.float32)
    return sel.astype(ml_dtypes.float8_e4m3)


def _Ts_host(T, aTs):
    """Precontracted T for the S matmuls: Ts[i, o] = sum_k T[i, o, k] for
    o's whose tile holds relu values, else 0; packed for DoubleRow."""
    Ts = T.sum(axis=2) * aTs                      # [IN_FEAT, OUT_FEAT]
    mask = np.zeros(OUT_FEAT, dtype=np.float32)
    for t in RELU_T:
        mask[4 * t:4 * t + 4] = 1.0
    return _pack_pairs((Ts * mask).astype(ml_dtypes.float8_e4m3))


def _sel2_host():
    s = np.zeros((128, OUT_FEAT), dtype=np.float32)
    s[:OUT_FEAT, :] = np.eye(OUT_FEAT)
    s[OUT_FEAT:, :] = np.eye(OUT_FEAT)
    return s.astype(ml_dtypes.bfloat16)


def _xT_cols(c):
    """Column -> source batch-row map for core c: partners c+1..c+3 full at
    cols 0:192, then block c+4 (even slots) interleaved with the diagonal
    block c (odd slots)."""
    cols = []
    for s in range(3):
        b = (c + 1 + s) % 8
        cols.extend(range(64 * b, 64 * b + 64))
    b4 = (c + 4) % 8
    for m in range(64):
        cols.append(64 * b4 + m)
        cols.append(64 * c + m)
    return np.array(cols)


def _pack_pairs(a):
    """[512, n] -> [256, 2n]: infeat chunks (2q, 2q+1) interleaved in the
    free dim (the DoubleRow pair dimension)."""
    n = a.shape[1]
    out = np.empty((256, 2 * n), dtype=a.dtype)
    for q in range(2):
        out[q * 128:(q + 1) * 128, 0:n] = a[(2 * q) * 128:(2 * q + 1) * 128]
        out[q * 128:(q + 1) * 128, n:2 * n] = \
            a[(2 * q + 1) * 128:(2 * q + 2) * 128]
    return out


def _pack_pairs_tiled(a):
    """Like _pack_pairs but tile-contiguous: tile t (128 cols) occupies
    cols [t*256, (t+1)*256) with its pair halves side by side, so half
    the Tm DMA unlocks 8 M tiles."""
    n = a.shape[1]
    nt = n // 128
    out = np.empty((256, 2 * n), dtype=a.dtype)
    for q in range(2):
        for t in range(nt):
            out[q * 128:(q + 1) * 128, t * 256:t * 256 + 128] = \
                a[(2 * q) * 128:(2 * q + 1) * 128, t * 128:(t + 1) * 128]
            out[q * 128:(q + 1) * 128, t * 256 + 128:(t + 1) * 256] = \
                a[(2 * q + 1) * 128:(2 * q + 2) * 128,
                  t * 128:(t + 1) * 128]
    return out


def _in_maps(x, T):
    f8 = ml_dtypes.float8_e4m3
    ax = _pow2_scale(float(np.abs(x).max()))
    aT = _pow2_scale(float(np.abs(T).max()))
    beta = 2.0 ** 12 / (ax * aT)
    T2 = T.reshape(IN_FEAT, OK)
    Tb = _pack_pairs_tiled((T2 * aT).astype(f8))
    Tsum = T2.reshape(IN_FEAT, OUT_FEAT, KERNEL_DIM)
    aTs = _pow2_scale(float(np.abs(Tsum.sum(axis=2)).max()))
    Tsb = _Ts_host(Tsum, aTs)
    betaS = 2.0 ** 12 / (aTs * ax)
    selb = _selb_host(beta)
    seldr = _seldr_host(beta, betaS)
    sel2b = _sel2_host()
    nbeta = np.full((OUT_FEAT, 1), -betaS, dtype=np.float32)
    xT = np.ascontiguousarray(x.T) * ax
    maps = []
    for c in range(N_CORES):
        xTc = np.ascontiguousarray(xT[:, _xT_cols(c)])
        maps.append({"xT": _pack_pairs(xTc.astype(f8)), "Tm": Tb,
                     "Ts": Tsb, "selb": selb, "seldr": seldr,
                     "sel2": sel2b, "nbeta": nbeta})
    return maps


def _assemble(x, results):
    """results: list of 8 dicts with 'rowS' [o, i] and 'accS' [o, 320]."""
    mbd = np.zeros((BATCH, OUT_FEAT), dtype=np.float32)
    for c in range(N_CORES):
        rs = np.concatenate([np.asarray(results[c]["rowsA"], np.float32),
                             np.asarray(results[c]["rowsB"], np.float32)],
                            axis=1)
        acc = sum(np.asarray(results[c][f"acc{ch}"], np.float32)
                  for ch in range(ROWB // CHUNK))
        # own rows: row sums (j <= i, incl. the self term exp(0)=1 and the
        # c+4 pair-diagonal) + strict diagonal transpose tail (j > i) from
        # the odd interleave slots
        mbd[64 * c:64 * (c + 1), :] += rs.T + acc[:, PBASE + 1::2].T
        # full partner transposes c+1..c+3
        for s in range(3):
            b = (c + 1 + s) % 8
            mbd[64 * b:64 * (b + 1), :] += acc[:, 64 * s:64 * (s + 1)].T
        # half-partner transpose: block c+4's j > i tail from the even
        # interleave slots (strict, pair-diagonal counted once per side)
        b4 = (c + 4) % 8
        mbd[64 * b4:64 * (b4 + 1), :] += acc[:, PBASE::2].T
    mbd -= 1.0  # reference subtracts the self-similarity exp(0)=1
    return np.concatenate([np.asarray(x, np.float32), mbd], axis=1)


def kernel(x, T):
    from concourse import bass_utils

    x = np.asarray(x, dtype=np.float32)
    T = np.asarray(T, dtype=np.float32)

    if "nc" not in _cache:
        _cache["nc"] = _build_nc()
    nc = _cache["nc"]

    res = bass_utils.run_bass_kernel_spmd(
        nc, _in_maps(x, T), core_ids=list(range(N_CORES)))
    return _assemble(x, res.results)
